# revision 4
# baseline (speedup 1.0000x reference)
"""Trainium2 Bass kernel for nn_ConvertParamMomentsTEtoParamsTE.

Math (per batch sample b):
    vinv   = 1/varh_diag                       [64]
    inner1 = vinv*muh
    R1     = -inner1
    R2     = vinv*varh_diagTE*vinv*muh - vinv*muhTE
    c1     = -(vinv*varh_diagTE*vinv)
    wtTE   = vinv[:,None]*varvhTE + c1[:,None]*varvh            [64,128]
    bTE    = muvTE + sum_h(R1[:,None]*varvhTE + R2[:,None]*varvh)   [128]
    tr     = sum_{h,v} varvh * (2*vinv[:,None]*varvhTE + c1[:,None]*varvh)
    sig2TE = (varvbarTE - tr)/nv

Distribution: pure data-parallel, batch 4096 split 512/core across 8 cores.

On-chip layout: flattened (b,h) rows on the 128 SBUF partitions (2 samples
per 128-row chunk), v on the free dim.  Per-(b,h) coefficients are per-
partition scalars.  The h-contraction for bTE runs on the TensorEngine as
[128,2]x[128,128] matmuls with block-diagonal coefficient matrices (one
column per sample in the chunk); muvTE is accumulated into the same PSUM
group via a [2,2] identity matmul so PSUM can be DMA'd straight to DRAM.
The trace reduction is a fused multiply + per-chunk free-dim reduce whose
[128] partials are contracted once at the end by a single ones-block-diag
matmul.
"""

import numpy as np

import concourse.bass as bass
import concourse.bacc as bacc
import concourse.mybir as mybir
import concourse.tile as tile
from concourse.bass_utils import run_bass_kernel_spmd
from concourse.masks import make_identity

B, NH, NV = 4096, 64, 128
NCORES = 8
BC = B // NCORES            # samples per core
NROWS = BC * NH             # flattened (b,h) rows per core
NCH = NROWS // 128          # 128-row chunks per core (256)
G = 4                       # chunks per big tile
NBT = NCH // G              # big tiles per core
F32 = mybir.dt.float32
AL = mybir.AluOpType
AF = mybir.ActivationFunctionType


def build_nc(inv_nv: float):
    """Build + compile the per-core Bass program (SPMD, identical on all cores)."""
    nc = bacc.Bacc("TRN2", target_bir_lowering=False, debug=False, num_devices=NCORES)

    muvTE = nc.dram_tensor("muvTE", [BC, NV], F32, kind="ExternalInput")
    varvhTE = nc.dram_tensor("varvhTE", [BC, NH, NV], F32, kind="ExternalInput")
    varh_diagTE = nc.dram_tensor("varh_diagTE", [BC, NH], F32, kind="ExternalInput")
    varh_diag = nc.dram_tensor("varh_diag", [BC, NH], F32, kind="ExternalInput")
    muh = nc.dram_tensor("muh", [BC, NH], F32, kind="ExternalInput")
    varvh = nc.dram_tensor("varvh", [BC, NH, NV], F32, kind="ExternalInput")
    muhTE = nc.dram_tensor("muhTE", [BC, NH], F32, kind="ExternalInput")
    varvbarTE = nc.dram_tensor("varvbarTE", [BC], F32, kind="ExternalInput")

    bTE = nc.dram_tensor("bTE", [BC, NV], F32, kind="ExternalOutput")
    wtTE = nc.dram_tensor("wtTE", [BC, NH, NV], F32, kind="ExternalOutput")
    sig2TE = nc.dram_tensor("sig2TE", [BC], F32, kind="ExternalOutput")

    def big_ap(t, T):
        # [128p, G, 128v] view of flat rows [T*G*128, (T+1)*G*128) of a
        # [NROWS, NV] row-major DRAM tensor: partition p = row within chunk.
        return bass.AP(t, T * G * 128 * NV, [[NV, 128], [128 * NV, G], [1, NV]])

    def bte_ap(t, T):
        # [2s, G, 128v] view of rows [T*2G, (T+1)*2G) of [BC, NV]: row = 2c+s.
        return bass.AP(t, T * 2 * G * NV, [[NV, 2], [2 * NV, G], [1, NV]])

    def pm_ap(t):
        # partition-major [128, NCH] view of a flat [NROWS] vector
        return bass.AP(t, 0, [[NCH, 128], [1, NCH]])

    def s_major_ap(t):
        # [2s, NCH] view of a [BC] vector: element (s, n) = v[2n+s]
        return bass.AP(t, 0, [[1, 2], [2, NCH]])

    with tile.TileContext(nc) as tc:
        with (
            tc.tile_pool(name="persist", bufs=1) as pp,
            tc.tile_pool(name="prep_ps", bufs=2, space="PSUM") as prep_ps,
            tc.tile_pool(name="big", bufs=3) as bp,
            tc.tile_pool(name="muvp", bufs=3) as muvp,
            tc.tile_pool(name="bte_ps", bufs=4, space="PSUM") as bte_ps,
        ):
            # ---- coefficient prep (partition-major layout) ----
            vd_pm = pp.tile([128, NCH], F32, tag="vd_pm")
            vdTE_pm = pp.tile([128, NCH], F32, tag="vdTE_pm")
            muh_pm = pp.tile([128, NCH], F32, tag="muh_pm")
            muhTE_pm = pp.tile([128, NCH], F32, tag="muhTE_pm")
            nc.sync.dma_start(out=vd_pm, in_=pm_ap(varh_diag))
            nc.sync.dma_start(out=vdTE_pm, in_=pm_ap(varh_diagTE))
            nc.sync.dma_start(out=muh_pm, in_=pm_ap(muh))
            nc.sync.dma_start(out=muhTE_pm, in_=pm_ap(muhTE))

            vinv_pm = pp.tile([128, NCH], F32, tag="vinv_pm")
            nc.vector.reciprocal(out=vinv_pm, in_=vd_pm)
            tmp_pm = pp.tile([128, NCH], F32, tag="tmp_pm")
            nc.vector.tensor_mul(out=tmp_pm, in0=vinv_pm, in1=vdTE_pm)
            inner1_pm = pp.tile([128, NCH], F32, tag="inner1_pm")
            nc.vector.tensor_mul(out=inner1_pm, in0=vinv_pm, in1=muh_pm)
            # R2 = tmp*inner1 - vinv*muhTE
            q2_pm = pp.tile([128, NCH], F32, tag="q2_pm")
            nc.vector.tensor_mul(out=q2_pm, in0=vinv_pm, in1=muhTE_pm)
            r2_pm = pp.tile([128, NCH], F32, tag="r2_pm")
            nc.vector.tensor_mul(out=r2_pm, in0=tmp_pm, in1=inner1_pm)
            nc.vector.tensor_sub(out=r2_pm, in0=r2_pm, in1=q2_pm)
            # c1 = -(tmp*vinv);  R1 = -inner1
            c1_pm = pp.tile([128, NCH], F32, tag="c1_pm")
            nc.vector.scalar_tensor_tensor(
                out=c1_pm, in0=tmp_pm, scalar=-1.0, in1=vinv_pm,
                op0=AL.mult, op1=AL.mult,
            )
            r1_pm = pp.tile([128, NCH], F32, tag="r1_pm")
            nc.vector.tensor_scalar_mul(out=r1_pm, in0=inner1_pm, scalar1=-1.0)

            # ---- transpose coefficients to chunk-major layout ----
            # pm[p, c] = coef[p*NCH + c]; target nm[p', n] = coef[n*128 + p'].
            # transpose(pm[:, half*128:(half+1)*128])[:, j] = nm[:, 2j+half]
            identity = pp.tile([128, 128], F32, tag="identity")
            make_identity(nc, identity)

            def to_nm(src_pm, tag):
                dst = pp.tile([128, NCH], F32, tag=tag)
                dst_v = dst.rearrange("p (n two) -> p n two", two=2)
                for half in range(2):
                    ps = prep_ps.tile([128, 128], F32, tag="tr_ps")
                    nc.tensor.transpose(
                        ps, src_pm[:, half * 128:(half + 1) * 128], identity
                    )
                    nc.scalar.copy(out=dst_v[:, :, half], in_=ps)
                return dst

            vinv_nm = to_nm(vinv_pm, "vinv_nm")
            c1_nm = to_nm(c1_pm, "c1_nm")
            r1_nm = to_nm(r1_pm, "r1_nm")
            r2_nm = to_nm(r2_pm, "r2_nm")

            # ---- block-diagonal matmul coefficient matrices ----
            # coefX_all[:, 2n:2n+2] is the [128, 2] lhsT for chunk n:
            # column s holds the coefficient on partitions [64s, 64s+64), else 0.
            def to_blockdiag(src_nm, tag):
                dst = pp.tile([128, 2 * NCH], F32, tag=tag)
                nc.vector.memset(dst, 0.0)
                dst_v = dst.rearrange("p (n two) -> p n two", two=2)
                nc.scalar.copy(out=dst_v[0:64, :, 0], in_=src_nm[0:64, :])
                nc.scalar.copy(out=dst_v[64:128, :, 1], in_=src_nm[64:128, :])
                return dst

            coefA = to_blockdiag(r1_nm, "coefA")
            coefB = to_blockdiag(r2_nm, "coefB")

            ones_bd = pp.tile([128, 2], F32, tag="ones_bd")
            nc.vector.memset(ones_bd, 0.0)
            nc.vector.memset(ones_bd[0:64, 0:1], 1.0)
            nc.vector.memset(ones_bd[64:128, 1:2], 1.0)

            id2 = pp.tile([2, 2], F32, tag="id2")
            make_identity(nc, id2)

            varvbar_sb = pp.tile([2, NCH], F32, tag="varvbar_sb")
            nc.sync.dma_start(out=varvbar_sb, in_=s_major_ap(varvbarTE))
            nc.scalar.mul(out=varvbar_sb, in_=varvbar_sb, mul=inv_nv)

            r_all = pp.tile([128, NCH], F32, tag="r_all")

            # ---- main loop over big tiles ----
            for T in range(NBT):
                vTE_t = bp.tile([128, G, NV], F32, tag="vTE")
                vvh_t = bp.tile([128, G, NV], F32, tag="vvh")
                nc.sync.dma_start(out=vTE_t, in_=big_ap(varvhTE, T))
                nc.sync.dma_start(out=vvh_t, in_=big_ap(varvh, T))
                muv_t = muvp.tile([2, G, NV], F32, tag="muv")
                nc.sync.dma_start(out=muv_t, in_=bte_ap(muvTE, T))

                t2_t = bp.tile([128, G, NV], F32, tag="t2")
                wt_t = bp.tile([128, G, NV], F32, tag="wt")
                for g in range(G):
                    n = T * G + g
                    # t2 = c1 * varvh      (ScalarEngine)
                    nc.scalar.activation(
                        out=t2_t[:, g, :], in_=vvh_t[:, g, :], func=AF.Copy,
                        scale=c1_nm[:, n:n + 1],
                    )
                    # wt = vinv*varvhTE + t2   (VectorEngine)
                    nc.vector.scalar_tensor_tensor(
                        out=wt_t[:, g, :], in0=vTE_t[:, g, :],
                        scalar=vinv_nm[:, n:n + 1], in1=t2_t[:, g, :],
                        op0=AL.mult, op1=AL.add,
                    )
                nc.sync.dma_start(out=big_ap(wtTE, T), in_=wt_t)

                # u = 2*wt - t2 = 2*vinv*varvhTE + c1*varvh
                u_t = bp.tile([128, G, NV], F32, tag="u")
                uf = u_t.rearrange("p g v -> p (g v)")
                nc.vector.scalar_tensor_tensor(
                    out=uf, in0=wt_t.rearrange("p g v -> p (g v)"), scalar=2.0,
                    in1=t2_t.rearrange("p g v -> p (g v)"),
                    op0=AL.mult, op1=AL.subtract,
                )
                # I = u * varvh  (GpSimd), then per-chunk row sums (VectorEngine)
                i_t = bp.tile([128, G, NV], F32, tag="i")
                nc.gpsimd.tensor_mul(
                    out=i_t.rearrange("p g v -> p (g v)"), in0=uf,
                    in1=vvh_t.rearrange("p g v -> p (g v)"),
                )
                nc.vector.tensor_reduce(
                    out=r_all[:, T * G:(T + 1) * G], in_=i_t,
                    axis=mybir.AxisListType.X, op=AL.add,
                )

                # bTE chunk: psum[s, v] = sum_k coefA[k,s]*varvhTE[k,v]
                #   + sum_k coefB[k,s]*varvh[k,v] + muvTE[2n+s, v]
                ps = bte_ps.tile([2, G, NV], F32, tag="bte")
                for g in range(G):
                    n = T * G + g
                    nc.tensor.matmul(
                        ps[:, g, :], coefA[:, 2 * n:2 * n + 2], vTE_t[:, g, :],
                        start=True, stop=False,
                    )
                    nc.tensor.matmul(
                        ps[:, g, :], coefB[:, 2 * n:2 * n + 2], vvh_t[:, g, :],
                        start=False, stop=False,
                    )
                    nc.tensor.matmul(
                        ps[:, g, :], id2, muv_t[:, g, :],
                        start=False, stop=True,
                    )
                bte_sb = muvp.tile([2, G, NV], F32, tag="bte_sb")
                nc.scalar.copy(out=bte_sb, in_=ps)
                nc.sync.dma_start(out=bte_ap(bTE, T), in_=bte_sb)

            # ---- trace epilogue ----
            tr_ps = prep_ps.tile([2, NCH], F32, tag="tr_mm")
            nc.tensor.matmul(tr_ps, ones_bd, r_all, start=True, stop=True)
            sig2_sb = pp.tile([2, NCH], F32, tag="sig2_sb")
            # stt reads PSUM tr_ps directly; result lands in SBUF for the DMA
            # sig2 = varvbar/nv - tr/nv   (varvbar_sb is pre-scaled by 1/nv)
            nc.vector.scalar_tensor_tensor(
                out=sig2_sb, in0=tr_ps, scalar=-inv_nv, in1=varvbar_sb,
                op0=AL.mult, op1=AL.add,
            )
            nc.sync.dma_start(out=s_major_ap(sig2TE), in_=sig2_sb)

    nc.compile()
    return nc


_NC_CACHE: dict[float, object] = {}


def _get_nc(inv_nv: float):
    if inv_nv not in _NC_CACHE:
        _NC_CACHE[inv_nv] = build_nc(inv_nv)
    return _NC_CACHE[inv_nv]


_IN_NAMES = (
    "muvTE", "varvhTE", "varh_diagTE", "varh_diag", "muh", "varvh",
    "muhTE", "varvbarTE",
)


def make_in_maps(inputs: dict) -> list[dict]:
    arrs = {k: np.ascontiguousarray(np.asarray(inputs[k], dtype=np.float32))
            for k in _IN_NAMES}
    in_maps = []
    for i in range(NCORES):
        sl = slice(i * BC, (i + 1) * BC)
        in_maps.append({k: np.ascontiguousarray(v[sl]) for k, v in arrs.items()})
    return in_maps


def kernel(**inputs):
    nv = float(np.asarray(inputs["nv"]))
    nc = _get_nc(1.0 / nv)
    in_maps = make_in_maps(inputs)
    res = run_bass_kernel_spmd(nc, in_maps, core_ids=list(range(NCORES)))
    bTE = np.concatenate([res.results[i]["bTE"] for i in range(NCORES)], axis=0)
    wtTE = np.concatenate([res.results[i]["wtTE"] for i in range(NCORES)], axis=0)
    sig2TE = np.concatenate([res.results[i]["sig2TE"] for i in range(NCORES)], axis=0)
    muhTE = np.asarray(inputs["muhTE"], dtype=np.float32)
    varh_diagTE = np.asarray(inputs["varh_diagTE"], dtype=np.float32)
    return bTE, wtTE, sig2TE, muhTE, varh_diagTE
